# revision 22
# baseline (speedup 1.0000x reference)
"""Trainium2 Bass kernel for a dense transformer block (B=2, T=2048, C=1024,
H=16, D=64, FF=4096), SPMD on 8 NeuronCores.

Sharding: pure data-parallel over tokens, zero collectives.
  core cid -> batch b = cid // 4, rank r = cid % 4.
  Each batch's 2048 tokens split into 8 causal chunks of 256; rank r owns
  chunks {r, 7-r} (zigzag balances causal attention work across ranks).
  Each core redundantly computes LN1 + K + V for its whole batch (cheaper
  than the slow on-chip ring collectives), then attention, projection, LN2
  and the MLP for its own 512 tokens only.

Layouts: matmuls keep activations transposed ([c, t]: contraction dim on
partitions); LayerNorm runs in [t, c] (free-dim reductions); PE transposes
convert. K and V stream through DRAM bounce buffers ([c, t] resp. [t, h*65]
with a ones column per head for softmax row-sums); the 1/Z partition
broadcast is a K=1 matmul. LN gamma/beta and the 1/sqrt(D) score scale are
folded into weights host-side. All matmuls run as float32r (full-rate fp32
storage mode).

One NEFF runs on all 8 cores, so causal ranges must be data-independent:
attention runs a fixed union schedule (q-half 0: s-chunks 0..3, q-half 1:
s-chunks 0..7) and host-fed per-core 0/1 masks gate inactive chunks and the
diagonal triangle. q-half 1 is always the late chunk (>= 4), so its s-chunks
0..3 are unconditionally active and skip the mask multiply.
"""

import numpy as np

B, T, C = 2, 2048, 1024
H, D = 16, 64
FF = 4 * C
EPS = 1e-6
N_CORES = 8
NCHUNK = 8
CH = T // NCHUNK        # 256 tokens per causal chunk
RANKS = 4
OWN = T // RANKS        # 512 tokens owned per core
P = 128
NB = 512                # matmul moving-dim tile
KC = C // P             # 8 contraction chunks over C
TB = T // NB            # 4 column blocks over T
FB = FF // P            # 32 ff row blocks
VW = H * (D + 1)        # v1 row width (ones column per head)

MASKED_PAIRS = [(0, sc) for sc in range(4)] + [(1, sc) for sc in range(4, 8)]
MASKED_SET = set(MASKED_PAIRS)
SRANGE = (RANKS, NCHUNK)  # union s-chunk counts per q-half


def build_core_program(nc, tile, mybir, n_iters=1):
    from contextlib import ExitStack
    from concourse import masks as masks_mod

    dt = mybir.dt
    f32 = dt.float32
    f32r = dt.float32r
    AF = mybir.ActivationFunctionType
    ALU = mybir.AluOpType
    AX = mybir.AxisListType

    x_full = nc.dram_tensor("x_full", [T, C], f32, kind="ExternalInput").ap()
    x_own = nc.dram_tensor("x_own", [OWN, C], f32, kind="ExternalInput").ap()
    xT_own = nc.dram_tensor("xT_own", [C, OWN], f32, kind="ExternalInput").ap()
    wqkvT = nc.dram_tensor("wqkvT", [C, 3 * C], f32r, kind="ExternalInput").ap()
    wprojT = nc.dram_tensor("wprojT", [C, C], f32r, kind="ExternalInput").ap()
    wl1T = nc.dram_tensor("wl1T", [C, FF], f32r, kind="ExternalInput").ap()
    wl3T = nc.dram_tensor("wl3T", [FF, C], f32r, kind="ExternalInput").ap()
    bqkv = nc.dram_tensor("bqkv", [3 * C], f32, kind="ExternalInput").ap()
    bproj = nc.dram_tensor("bproj", [C], f32, kind="ExternalInput").ap()
    bl1 = nc.dram_tensor("bl1", [FF], f32, kind="ExternalInput").ap()
    bl3 = nc.dram_tensor("bl3", [C], f32, kind="ExternalInput").ap()
    mask_in = nc.dram_tensor(
        "mask", [len(MASKED_PAIRS), 2, P, CH], f32r, kind="ExternalInput"
    ).ap()
    outT = nc.dram_tensor("outT", [C, OWN], f32, kind="ExternalOutput").ap()

    with tile.TileContext(nc) as tc, ExitStack() as ctx:
        dramp = ctx.enter_context(
            tc.tile_pool(name="dramb", bufs=1, space="DRAM")
        )
        v1_d = dramp.tile([T, VW], f32r, name="v1_d")
        kT_d = dramp.tile([C, T], f32r, name="kT_d")
        qT_d = dramp.tile([C, OWN], f32r, name="qT_d")
        cpool = ctx.enter_context(tc.tile_pool(name="const", bufs=1))
        identity = cpool.tile([P, P], f32, name="identity")
        masks_mod.make_identity(nc, identity[:])
        onesf = cpool.tile([P, P], f32, name="onesf")
        nc.vector.memset(onesf[:], 1.0)
        ones1 = cpool.tile([1, D], f32r, name="ones1")
        nc.scalar.activation(ones1[:], onesf[0:1, 0:D], AF.Identity)
        ones16 = cpool.tile([P, H], f32r, name="ones16")
        nc.scalar.activation(ones16[:], onesf[:, 0:H], AF.Identity)
        ones128 = cpool.tile([1, P], f32r, name="ones128")
        nc.scalar.activation(ones128[:], onesf[0:1, :], AF.Identity)

        bqkv_t = cpool.tile([P, 3 * KC], f32, name="bqkv_t")
        nc.sync.dma_start(bqkv_t[:], bqkv.rearrange("(j p) -> p j", p=P))
        bq64 = cpool.tile([D, H], f32, name="bq64")
        nc.sync.dma_start(bq64[:], bqkv[0:C].rearrange("(j p) -> p j", p=D))
        bproj_t = cpool.tile([P, KC], f32, name="bproj_t")
        nc.sync.dma_start(bproj_t[:], bproj.rearrange("(j p) -> p j", p=P))
        bl1_t = cpool.tile([P, FB], f32, name="bl1_t")
        nc.sync.dma_start(bl1_t[:], bl1.rearrange("(j p) -> p j", p=P))
        bl3_t = cpool.tile([P, KC], f32, name="bl3_t")
        nc.sync.dma_start(bl3_t[:], bl3.rearrange("(j p) -> p j", p=P))

        stat = ctx.enter_context(tc.tile_pool(name="stat", bufs=4))
        wpool = ctx.enter_context(tc.tile_pool(name="w", bufs=14))
        evp = ctx.enter_context(tc.tile_pool(name="ev", bufs=4))
        psum = ctx.enter_context(tc.tile_pool(name="ps", bufs=3, space="PSUM"))
        pst = ctx.enter_context(tc.tile_pool(name="pst", bufs=2, space="PSUM"))
        pav = ctx.enter_context(tc.tile_pool(name="pav", bufs=2, space="PSUM"))
        pbc = ctx.enter_context(tc.tile_pool(name="pbc", bufs=1, space="PSUM"))

        def mm(out, lhsT, rhs, **kw):
            nc.tensor.matmul(out, lhsT, rhs, **kw)

        # v-bias broadcast tile [P, C] (bias varies along the free dim in the
        # [t, vcol] layout, so per-partition ACT bias can't apply it)
        bv_row = cpool.tile([1, C], f32r, name="bv_row")
        nc.gpsimd.dma_start(
            bv_row[:], bqkv[2 * C:3 * C].rearrange("(a c) -> a c", a=1)
        )
        bvb = cpool.tile([P, C], f32, name="bvb")
        for j in range(2):
            ps = psum.tile([P, NB], f32, name="ps")
            mm(ps[:], ones128[:], bv_row[:, j * NB:(j + 1) * NB],
               start=True, stop=True)
            nc.scalar.copy(bvb[:, j * NB:(j + 1) * NB], ps[:])

        def layernorm_tile(xt, z, work):
            s = stat.tile([P, 1], f32, name="s")
            nc.vector.reduce_sum(s[:], xt[:], axis=AX.X)
            nmu = stat.tile([P, 1], f32, name="nmu")
            nc.scalar.mul(nmu[:], s[:], -1.0 / C)
            sq = work.tile([P, C], f32, name="sq")
            nc.scalar.activation(sq[:], xt[:], AF.Square, bias=nmu[:, 0:1])
            ssq = stat.tile([P, 1], f32, name="ssq")
            nc.vector.reduce_sum(ssq[:], sq[:], axis=AX.X)
            sd = stat.tile([P, 1], f32, name="sd")
            nc.scalar.activation(sd[:], ssq[:], AF.Sqrt, scale=1.0 / (C - 1))
            sde = stat.tile([P, 1], f32, name="sde")
            nc.vector.tensor_scalar_add(sde[:], sd[:], EPS)
            rs = stat.tile([P, 1], f32, name="rs")
            nc.vector.reciprocal(rs[:], sde[:])
            nc.vector.tensor_scalar(
                z[:], xt[:], nmu[:, 0:1], rs[:, 0:1], ALU.add, ALU.mult
            )

        def transpose_to(dst_tiles, src_tile, dst_col):
            for j in range(KC):
                ps = pst.tile([P, P], f32, name="tps")
                nc.tensor.transpose(
                    ps[:], src_tile[:, j * P:(j + 1) * P], identity[:]
                )
                nc.scalar.copy(dst_tiles[j][:, dst_col:dst_col + P], ps[:])

        def ln_transpose(src_dram, row0, dst_tiles, dst_col, work):
            xt = work.tile([P, C], f32, name="xt")
            nc.sync.dma_start(xt[:], src_dram[row0:row0 + P, :])
            z = work.tile([P, C], f32, name="z")
            layernorm_tile(xt, z, work)
            transpose_to(dst_tiles, z, dst_col)

        def body(ctx2):

            # ---- A-own + D: LN1 on own 512 tokens -> lnq; Q -> qT ----
            with tc.tile_pool(name="lnqp", bufs=1) as lnqp, \
                 tc.tile_pool(name="workq", bufs=2) as workq:
                lnq = [lnqp.tile([P, OWN], f32r, name=f"lnq{j}")
                       for j in range(KC)]
                for i in range(OWN // P):
                    ln_transpose(x_own, i * P, lnq, i * P, workq)
                for m in range(H):
                    wq = [wpool.tile([P, D], f32r, name="wq", tag="wsm")
                          for _ in range(KC)]
                    for kc in range(KC):
                        nc.sync.dma_start(
                            wq[kc][:],
                            wqkvT[kc * P:(kc + 1) * P, m * D:(m + 1) * D],
                        )
                    ps = psum.tile([P, NB], f32, name="ps")
                    for kc in range(KC):
                        mm(ps[0:D, :], wq[kc][:], lnq[kc][:],
                           start=(kc == 0), stop=(kc == KC - 1))
                    qe = evp.tile([D, OWN], f32r, name="qe")
                    nc.scalar.activation(
                        qe[:], ps[0:D, :], AF.Identity,
                        bias=bq64[:, m:m + 1],
                    )
                    nc.sync.dma_start(qT_d[m * D:(m + 1) * D, :], qe[:])

            # ---- A-full + B + C: LN1 all tokens; V -> v1_d; K -> kT_d ----
            with tc.tile_pool(name="ln1p", bufs=1) as ln1p, \
                 tc.tile_pool(name="worka", bufs=2) as worka:
                ln1T = [ln1p.tile([P, T], f32r, name=f"ln1T{j}")
                        for j in range(KC)]
                for i in range(T // P):
                    ln_transpose(x_full, i * P, ln1T, i * P, worka)

                v1_v = v1_d.rearrange("t (h w) -> t h w", h=H)
                for i in range(T // P):
                    nc.sync.dma_start(v1_v[i * P:(i + 1) * P, :, D],
                                      ones16[:])
                with tc.tile_pool(name="wvp", bufs=10) as wvp:
                    for nb in range(2):
                        wv = [wvp.tile([P, NB], f32r, name="wv")
                              for _ in range(KC)]
                        for kc in range(KC):
                            nc.sync.dma_start(
                                wv[kc][:],
                                wqkvT[kc * P:(kc + 1) * P,
                                      2 * C + nb * NB: 2 * C + (nb + 1) * NB],
                            )
                        for tb in range(T // P):
                            ps = psum.tile([P, NB], f32, name="ps")
                            for kc in range(KC):
                                mm(ps[:], ln1T[kc][:, tb * P:(tb + 1) * P],
                                   wv[kc][:],
                                   start=(kc == 0), stop=(kc == KC - 1))
                            ev = evp.tile([P, NB], f32r, name="evr")
                            nc.vector.tensor_add(
                                ev[:], ps[:], bvb[:, nb * NB:(nb + 1) * NB]
                            )
                            nc.sync.dma_start(
                                v1_v[tb * P:(tb + 1) * P,
                                     nb * 8:(nb + 1) * 8, 0:D],
                                ev.rearrange("p (h d) -> p h d", h=8),
                            )

                for m in range(KC):
                    wk = [wpool.tile([P, P], f32r, name="wk", tag="wsm")
                          for _ in range(KC)]
                    for kc in range(KC):
                        nc.sync.dma_start(
                            wk[kc][:],
                            wqkvT[kc * P:(kc + 1) * P,
                                  C + m * P: C + (m + 1) * P],
                        )
                    for nb in range(TB):
                        ps = psum.tile([P, NB], f32, name="ps")
                        for kc in range(KC):
                            mm(ps[:], wk[kc][:],
                               ln1T[kc][:, nb * NB:(nb + 1) * NB],
                               start=(kc == 0), stop=(kc == KC - 1))
                        ev = evp.tile([P, NB], f32r, name="evr")
                        nc.scalar.activation(
                            ev[:], ps[:], AF.Identity,
                            bias=bqkv_t[:, KC + m: KC + m + 1],
                        )
                        nc.sync.dma_start(
                            kT_d[m * P:(m + 1) * P, nb * NB:(nb + 1) * NB],
                            ev[:],
                        )

            # ---- F-pool (x1T lives F..H) wraps E..H ----
            x1Tp = ctx2.enter_context(tc.tile_pool(name="x1Tp", bufs=1))
            x1T = [x1Tp.tile([P, OWN], f32, name=f"x1T{j}") for j in range(KC)]

            # ---- E: attention -> aT; F: proj + residual + LN2 ----
            with tc.tile_pool(name="aTp", bufs=1) as aTp:
                aT = [aTp.tile([P, OWN], f32r, name=f"aT{j}")
                      for j in range(KC)]
                with tc.tile_pool(name="maskp", bufs=1) as mpool, \
                     tc.tile_pool(name="kvp", bufs=34) as kvp:
                    mtiles = {}
                    for i, (qh, sc) in enumerate(MASKED_PAIRS):
                        for sb in range(2):
                            mt = mpool.tile([P, CH], f32r,
                                            name=f"m{qh}_{sc}_{sb}")
                            nc.sync.dma_start(mt[:], mask_in[i, sb])
                            mtiles[(qh, sc, sb)] = mt

                    for h in range(H):
                        qt = kvp.tile([D, OWN], f32r, name="qt", bufs=3)
                        nc.sync.dma_start(qt[:], qT_d[h * D:(h + 1) * D, :])
                        kt_t, v1_t = {}, {}
                        for sc in range(NCHUNK):
                            for sb in range(2):
                                s0 = sc * CH + sb * P
                                kt = kvp.tile([D, P], f32r, name="kt")
                                nc.sync.dma_start(
                                    kt[:],
                                    kT_d[h * D:(h + 1) * D, s0:s0 + P],
                                )
                                kt_t[(sc, sb)] = kt
                                vt = kvp.tile([P, D + 1], f32r, name="vt")
                                nc.sync.dma_start(
                                    vt[:],
                                    v1_d[s0:s0 + P,
                                         h * (D + 1):(h + 1) * (D + 1)],
                                )
                                v1_t[(sc, sb)] = vt
                        for qh in range(2):
                            av = pav.tile([D + 1, CH], f32, name="av")
                            nsc = SRANGE[qh]
                            for sc in range(nsc):
                                masked = (qh, sc) in MASKED_SET
                                ex_blocks = []
                                for sb in range(2):
                                    ps = psum.tile([P, NB], f32, name="ps")
                                    mm(ps[:, 0:CH], kt_t[(sc, sb)][:],
                                       qt[:, qh * CH:(qh + 1) * CH],
                                       start=True, stop=True)
                                    ex = evp.tile([P, CH], f32r, name="ex")
                                    nc.scalar.activation(ex[:], ps[:, 0:CH],
                                                         AF.Exp)
                                    if masked:
                                        exm = evp.tile([P, CH], f32r,
                                                       name="exm")
                                        nc.gpsimd.tensor_mul(
                                            exm[:], ex[:],
                                            mtiles[(qh, sc, sb)][:],
                                        )
                                        ex_blocks.append(exm)
                                    else:
                                        ex_blocks.append(ex)
                                for sb in range(2):
                                    mm(av[:], v1_t[(sc, sb)][:],
                                       ex_blocks[sb][:],
                                       start=(sc == 0 and sb == 0),
                                       stop=(sc == nsc - 1 and sb == 1))
                            rz = stat.tile([1, CH], f32, name="rz")
                            nc.vector.reciprocal(rz[:], av[D:D + 1, :])
                            rzr = stat.tile([1, CH], f32r, name="rzr")
                            nc.scalar.activation(rzr[:], rz[:], AF.Identity)
                            bc = pbc.tile([D, CH], f32, name="bc")
                            mm(bc[:], ones1[:], rzr[:], start=True, stop=True)
                            bcs = evp.tile([D, CH], f32, name="bcs")
                            nc.vector.tensor_copy(bcs[:], bc[:])
                            nc.vector.tensor_mul(
                                aT[h // 2][(h % 2) * D:(h % 2 + 1) * D,
                                           qh * CH:(qh + 1) * CH],
                                av[0:D, :], bcs[:],
                            )

                # ---- F: proj + residual -> x1T ----
                for m in range(KC):
                    wp = [wpool.tile([P, P], f32r, name="wp", tag="wsm")
                          for _ in range(KC)]
                    for kc in range(KC):
                        nc.sync.dma_start(
                            wp[kc][:],
                            wprojT[kc * P:(kc + 1) * P, m * P:(m + 1) * P],
                        )
                    ps = psum.tile([P, NB], f32, name="ps")
                    for kc in range(KC):
                        mm(ps[:], wp[kc][:], aT[kc][:],
                           start=(kc == 0), stop=(kc == KC - 1))
                    ev = evp.tile([P, NB], f32, name="ev")
                    nc.scalar.activation(ev[:], ps[:], AF.Identity,
                                         bias=bproj_t[:, m:m + 1])
                    xo = evp.tile([P, NB], f32, name="xo", tag="ev")
                    nc.sync.dma_start(xo[:], xT_own[m * P:(m + 1) * P, :])
                    nc.vector.tensor_add(x1T[m][:], ev[:], xo[:])

            # ---- F2/G/H: LN2, MLP ----
            with tc.tile_pool(name="hTp", bufs=1) as hTp:
                hT = [hTp.tile([P, OWN], f32r, name=f"hT{j}")
                      for j in range(FB)]
                with tc.tile_pool(name="ln2p", bufs=1) as ln2p, \
                     tc.tile_pool(name="workf", bufs=2) as workf:
                    ln2T = [ln2p.tile([P, OWN], f32r, name=f"ln2T{j}")
                            for j in range(KC)]
                    for i in range(OWN // P):
                        x1 = workf.tile([P, C], f32, name="xt")
                        for j in range(KC):
                            ps = pst.tile([P, P], f32, name="tps")
                            nc.tensor.transpose(
                                ps[:], x1T[j][:, i * P:(i + 1) * P],
                                identity[:],
                            )
                            nc.scalar.copy(x1[:, j * P:(j + 1) * P], ps[:])
                        z = workf.tile([P, C], f32, name="z")
                        layernorm_tile(x1, z, workf)
                        transpose_to(ln2T, z, i * P)

                    for m in range(FB):
                        w1 = [wpool.tile([P, P], f32r, name="w1", tag="wsm")
                              for _ in range(KC)]
                        for kc in range(KC):
                            nc.sync.dma_start(
                                w1[kc][:],
                                wl1T[kc * P:(kc + 1) * P, m * P:(m + 1) * P],
                            )
                        ps = psum.tile([P, NB], f32, name="ps")
                        for kc in range(KC):
                            mm(ps[:], w1[kc][:], ln2T[kc][:],
                               start=(kc == 0), stop=(kc == KC - 1))
                        nc.scalar.activation(hT[m][:], ps[:], AF.Relu,
                                             bias=bl1_t[:, m:m + 1])

                with tc.tile_pool(name="w3p", bufs=36) as w3p:
                    for m in range(KC):
                        w3 = [w3p.tile([P, P], f32r, name="w3")
                              for _ in range(FB)]
                        for fc in range(FB):
                            nc.sync.dma_start(
                                w3[fc][:],
                                wl3T[fc * P:(fc + 1) * P, m * P:(m + 1) * P],
                            )
                        ps = psum.tile([P, NB], f32, name="ps")
                        for fc in range(FB):
                            mm(ps[:], w3[fc][:], hT[fc][:],
                               start=(fc == 0), stop=(fc == FB - 1))
                        ev = evp.tile([P, NB], f32, name="ev")
                        nc.scalar.activation(ev[:], ps[:], AF.Identity,
                                             bias=bl3_t[:, m:m + 1])
                        o = evp.tile([P, NB], f32, name="o", tag="ev")
                        nc.vector.tensor_add(o[:], ev[:], x1T[m][:])
                        nc.sync.dma_start(outT[m * P:(m + 1) * P, :], o[:])

        if n_iters == 1:
            with ExitStack() as ctx2:
                body(ctx2)
        else:
            with tc.For_i(0, n_iters, 1):
                with ExitStack() as ctx2:
                    body(ctx2)

    return nc


def _host_prep(x, qkv_w, proj_w, proj_b, l1_w, l1_b, l3_w, l3_b,
               ln1_g, ln1_b, ln2_g, ln2_b):
    f = np.float32
    x = np.asarray(x, f)
    qkv_w = np.asarray(qkv_w, f)
    scale = np.float32(D ** -0.5)
    w_eff = qkv_w * np.asarray(ln1_g, f)[None, :]
    b_eff = (qkv_w @ np.asarray(ln1_b, f)).astype(f)
    w_eff[:C] *= scale
    b_eff[:C] *= scale
    l1_eff = np.asarray(l1_w, f) * np.asarray(ln2_g, f)[None, :]
    bl1_eff = (np.asarray(l1_b, f)
               + np.asarray(l1_w, f) @ np.asarray(ln2_b, f)).astype(f)
    shared = {
        "wqkvT": np.ascontiguousarray(w_eff.T),
        "bqkv": b_eff,
        "wprojT": np.ascontiguousarray(np.asarray(proj_w, f).T),
        "bproj": np.asarray(proj_b, f),
        "wl1T": np.ascontiguousarray(l1_eff.T),
        "bl1": bl1_eff,
        "wl3T": np.ascontiguousarray(np.asarray(l3_w, f).T),
        "bl3": np.asarray(l3_b, f),
    }

    in_maps = []
    for cid in range(N_CORES):
        b, r = divmod(cid, RANKS)
        lo, hi = r, NCHUNK - 1 - r
        own_idx = np.r_[lo * CH:(lo + 1) * CH, hi * CH:(hi + 1) * CH]
        xb = x[b]
        x_own = np.ascontiguousarray(xb[own_idx])
        m = np.zeros((len(MASKED_PAIRS), 2, P, CH), f)
        tri0 = (np.arange(P)[:, None] <= np.arange(CH)[None, :]).astype(f)
        tri1 = (np.arange(P)[:, None] + P <= np.arange(CH)[None, :]).astype(f)
        for i, (qh, sc) in enumerate(MASKED_PAIRS):
            qc = lo if qh == 0 else hi
            if sc < qc:
                m[i] = 1.0
            elif sc == qc:
                m[i, 0] = tri0
                m[i, 1] = tri1
        in_maps.append({
            "x_full": np.ascontiguousarray(xb),
            "x_own": x_own,
            "xT_own": np.ascontiguousarray(x_own.T),
            "mask": m,
            **shared,
        })
    return in_maps


def _assemble(results):
    out = np.empty((B, T, C), np.float32)
    for cid in range(N_CORES):
        b, r = divmod(cid, RANKS)
        lo, hi = r, NCHUNK - 1 - r
        oT = results[cid]["outT"]
        out[b, lo * CH:(lo + 1) * CH] = oT[:, 0:CH].T
        out[b, hi * CH:(hi + 1) * CH] = oT[:, CH:2 * CH].T
    return out


_CACHE = {}


def get_nc(n_iters=1):
    if n_iters not in _CACHE:
        import concourse.bacc as bacc
        import concourse.tile as tile
        from concourse import mybir
        nc = bacc.Bacc("TRN2", target_bir_lowering=False, debug=False,
                       num_devices=N_CORES)
        build_core_program(nc, tile, mybir, n_iters=n_iters)
        nc.compile()
        _CACHE[n_iters] = nc
    return _CACHE[n_iters]


def run(inputs, n_iters=1):
    from concourse.bass_utils import run_bass_kernel_spmd
    in_maps = _host_prep(**inputs)
    nc = get_nc(n_iters)
    res = run_bass_kernel_spmd(nc, in_maps, list(range(N_CORES)))
    return _assemble(res.results)


def kernel(**inputs):
    return run(inputs, n_iters=1)
